# revision 1
# baseline (speedup 1.0000x reference)
"""Trainium2 Bass kernel for causal multi-head attention with adaptive
temperature (entropy-polynomial) softmax.

Problem shape: x [2, 2048, 1024], 16 heads x 64 dims, causal.
  q/k/v = x @ W{q,k,v}.T ; sim = q k^T / 8 (causal) ;
  attn = softmax(beta * sim), beta = f(entropy(softmax(sim))) ;
  out = (attn v) @ Wo.T + bo

Sharding (8 cores): core c owns batch b = c // 4 and heads
4*(c%4) .. 4*(c%4)+3.  Each core computes its heads' q/k/v projections
(tensor-parallel over the head dim), full [n, n] score tiles for its
heads, and a partial output projection over its 256 channel dims.
Host sums the 4 partials per batch and adds bo.

Device-side per core:
  phase A : qT/kT = (Wq/Wk slice) @ x^T  (f32r, qT pre-scaled by 1/8),
            v = x @ Wv_slice.T cast to bf16
  phase B1: entropy-stat sweep over all (row-block, head):
            scores chunk -> exp (accum Z1) -> l*exp(l) (accum D)
  stats   : one batched [128, 64] chain:
            H = ln Z1 - D/Z1 ; beta = where(H>.5, max(poly(H),1), 1)
  phase B2: rescore -> exp(beta*l) (accum Z2) -> normalize (bf16) ->
            DMA-xbar transpose -> attn^T @ v accumulation (bf16)
  phase C : partial = attn_out^T.T @ Wo_slice^T  (f32r)
"""

import numpy as np

import concourse.bass as bass
import concourse.tile as tile
from concourse import bacc, mybir
from concourse.bass_utils import run_bass_kernel_spmd
from concourse.masks import make_identity

F32 = mybir.dt.float32
F32R = mybir.dt.float32r
BF16 = mybir.dt.bfloat16
I32 = mybir.dt.int32
AFT = mybir.ActivationFunctionType
ALU = mybir.AluOpType

B, N, DIM = 2, 2048, 1024
H_TOT, HD = 16, 64
N_CORES = 8
NH = 4            # heads per core
CD = NH * HD      # 256 channel dims per core
NRB = N // 128    # 16 row blocks
NU = NRB * NH     # 64 (rb, head) units
POLY = [-0.037, 0.481, -2.3, 4.917, -1.791]
MASK_VAL = -1e30
SCALE = 1.0 / 8.0  # 1/sqrt(64)

CHUNK = 512          # score-chunk width (PSUM tile free size)
USE_DMA_T = True      # transpose attn via DMA xbar instead of PE


def _scores(nc, ps, q_l, kTm, base, off, cw):
    """matmul score chunk [128, cw] at row offset `off` into psum `ps`."""
    for o2 in range(0, cw, 512):
        sw = min(512, cw - o2)
        nc.tensor.matmul(ps[:, o2:o2 + sw], q_l,
                         kTm[base:base + 64, off + o2:off + o2 + sw],
                         start=True, stop=True)


def build_kernel():
    nc = bacc.Bacc("TRN2", target_bir_lowering=False, debug=False,
                   num_devices=N_CORES)

    xT = nc.dram_tensor("xT", [DIM, N], F32, kind="ExternalInput").ap()
    wqT = nc.dram_tensor("wqT", [DIM, CD], F32, kind="ExternalInput").ap()
    wkT = nc.dram_tensor("wkT", [DIM, CD], F32, kind="ExternalInput").ap()
    wvT = nc.dram_tensor("wvT", [DIM, CD], F32, kind="ExternalInput").ap()
    woT = nc.dram_tensor("woT", [CD, DIM], F32, kind="ExternalInput").ap()
    maskin = nc.dram_tensor("maskin", [128, 128], F32, kind="ExternalInput").ap()
    partial = nc.dram_tensor("partial", [N, DIM], F32, kind="ExternalOutput").ap()

    KC = DIM // 128  # 8 contraction chunks

    with tile.TileContext(nc) as tc:
        # ---- persistent pools (allocated first = live whole kernel) ----
        with tc.tile_pool(name="const", bufs=1) as constp, \
             tc.tile_pool(name="qkv_sb", bufs=1) as qkvp, \
             tc.tile_pool(name="attn_out", bufs=1) as aop, \
             tc.tile_pool(name="wo_sb", bufs=1) as wop, \
             tc.tile_pool(name="statsall", bufs=1) as sap:

            ident = constp.tile([128, 128], BF16)
            make_identity(nc, ident[:])
            mask = constp.tile([128, 128], F32)
            nc.sync.dma_start(mask[:], maskin[:])
            ones64 = constp.tile([128, NU], F32)
            nc.vector.memset(ones64[:], 1.0)

            # persistent activations
            qT = [qkvp.tile([128, N], F32R, tag=f"qT{m}", name=f"qT{m}") for m in range(2)]
            kT = [qkvp.tile([128, N], F32R, tag=f"kT{m}", name=f"kT{m}") for m in range(2)]
            v_bf = [qkvp.tile([128, CD], BF16, tag=f"v{j}", name=f"v{j}") for j in range(NRB)]
            attT = [aop.tile([128, N], F32R, tag=f"attT{m}", name=f"attT{m}") for m in range(2)]
            woS = [wop.tile([128, DIM], F32R, tag=f"wo{m}", name=f"wo{m}") for m in range(2)]

            Z1a = sap.tile([128, NU], F32)
            D1a = sap.tile([128, NU], F32)
            Z1p = sap.tile([128, 4 * NU], F32)
            D1p = sap.tile([128, 4 * NU], F32)
            beta_all = sap.tile([128, NU], F32)

            # ---- phase A: QKV projections ----
            with tc.tile_pool(name="xw_sb", bufs=1) as xwp, \
                 tc.tile_pool(name="qkv_ps", bufs=4, space="PSUM") as qkps:
                xTs = [xwp.tile([128, N], F32R, tag=f"xT{k}", name=f"xTs{k}") for k in range(KC)]
                wq_s = [xwp.tile([128, CD], F32R, tag=f"wq{k}", name=f"wq{k}") for k in range(KC)]
                wk_s = [xwp.tile([128, CD], F32R, tag=f"wk{k}", name=f"wk{k}") for k in range(KC)]
                wv_s = [xwp.tile([128, CD], F32R, tag=f"wv{k}", name=f"wv{k}") for k in range(KC)]
                # per-k interleave so the k=0 accumulation steps can start
                # after ~0.5 MB of DMA instead of after the whole 11 MB
                for k in range(KC):
                    sl = slice(128 * k, 128 * (k + 1))
                    nc.sync.dma_start(wq_s[k][:], wqT[sl, :].bitcast(F32R))
                    nc.sync.dma_start(wk_s[k][:], wkT[sl, :].bitcast(F32R))
                    nc.sync.dma_start(wv_s[k][:], wvT[sl, :].bitcast(F32R))
                    nc.sync.dma_start(xTs[k][:], xT[sl, :].bitcast(F32R))
                for m in range(2):
                    nc.sync.dma_start(woS[m][:], woT[128 * m:128 * (m + 1), :].bitcast(F32R))

                # qT / kT: [o, r] = sum_c W[o,c] x[r,c]
                # emit per head-pair (q then k) so attention on pair 0 can
                # start while pair 1 / v are still projecting
                for m in range(2):
                    for which, wt, dest, scl in (("q", wq_s, qT, SCALE), ("k", wk_s, kT, 1.0)):
                        for nn in range(N // 512):
                            pq = qkps.tile([128, 512], F32, tag="pq")
                            for k in range(KC):
                                nc.tensor.matmul(
                                    pq[:], wt[k][:, 128 * m:128 * (m + 1)],
                                    xTs[k][:, 512 * nn:512 * (nn + 1)],
                                    start=(k == 0), stop=(k == KC - 1))
                            nc.scalar.activation(
                                dest[m][:, 512 * nn:512 * (nn + 1)], pq[:],
                                AFT.Copy, bias=0.0, scale=scl)

                # v: [j, d] = sum_c xT[c,j] wvT[c,d]
                for jt in range(NRB):
                    pv = qkps.tile([128, CD], F32, tag="pv")
                    for k in range(KC):
                        nc.tensor.matmul(
                            pv[:], xTs[k][:, 128 * jt:128 * (jt + 1)], wv_s[k][:],
                            start=(k == 0), stop=(k == KC - 1))
                    nc.any.tensor_copy(v_bf[jt][:], pv[:])

            # ---- phase B ----
            with tc.tile_pool(name="scr", bufs=8) as scrp, \
                 tc.tile_pool(name="t2p", bufs=2) as t2p, \
                 tc.tile_pool(name="t2bfp", bufs=2) as t2bfp, \
                 tc.tile_pool(name="ttp", bufs=8) as ttp, \
                 tc.tile_pool(name="stats", bufs=4) as stp, \
                 tc.tile_pool(name="ost", bufs=2) as ostp:

                # ---- B1: entropy-stat sweep ----
                # per-chunk accumulators land directly in wide [128, 4*NU]
                # tiles; one batched 3D-AP reduce replaces the per-unit ones
                nc.vector.memset(Z1p[:], 0.0)
                nc.vector.memset(D1p[:], 0.0)
                with tc.tile_pool(name="b1_ps", bufs=8, space="PSUM") as scps:
                    for rb in range(NRB):
                        W = 128 * (rb + 1)
                        chunks = [(off, min(CHUNK, W - off)) for off in range(0, W, CHUNK)]
                        nck = len(chunks)
                        for h in range(NH):
                            col = rb * NH + h
                            m, base = h // 2, 64 * (h % 2)
                            q_l = qT[m][base:base + 64, 128 * rb:128 * (rb + 1)]
                            for ci, (off, cw) in enumerate(chunks):
                                ps = scps.tile([128, CHUNK], F32, tag="ps_s")
                                _scores(nc, ps, q_l, kT[m], base, off, cw)
                                if off + cw == W:
                                    nc.vector.tensor_tensor(
                                        out=ps[:, cw - 128:cw], in0=ps[:, cw - 128:cw],
                                        in1=mask[:], op=ALU.add)
                                t1 = scrp.tile([128, CHUNK], F32, tag="t1")
                                nc.scalar.activation(
                                    t1[:, :cw], ps[:, :cw], AFT.Exp,
                                    bias=0.0, scale=1.0,
                                    accum_out=Z1p[:, 4 * col + ci:4 * col + ci + 1])
                                s2 = scrp.tile([128, CHUNK], F32, tag="s2")
                                nc.vector.scalar_tensor_tensor(
                                    out=s2[:, :cw], in0=ps[:, :cw], scalar=1.0,
                                    in1=t1[:, :cw], op0=ALU.mult, op1=ALU.mult,
                                    accum_out=D1p[:, 4 * col + ci:4 * col + ci + 1])

                # ---- batched stats: H = ln Z1 - D/Z1 ; beta ----
                nc.vector.tensor_reduce(
                    out=Z1a[:], in_=Z1p.rearrange("p (u c) -> p u c", c=4),
                    axis=mybir.AxisListType.X, op=ALU.add)
                nc.vector.tensor_reduce(
                    out=D1a[:], in_=D1p.rearrange("p (u c) -> p u c", c=4),
                    axis=mybir.AxisListType.X, op=ALU.add)
                rz = stp.tile([128, NU], F32, tag="rz")
                nc.vector.reciprocal(rz[:], Z1a[:])
                dn = stp.tile([128, NU], F32, tag="dn")
                nc.vector.tensor_mul(dn[:], D1a[:], rz[:])
                lnz = stp.tile([128, NU], F32, tag="lnz")
                nc.scalar.activation(lnz[:], Z1a[:], AFT.Ln, bias=0.0, scale=1.0)
                Hent = stp.tile([128, NU], F32, tag="Hent")
                nc.vector.tensor_sub(Hent[:], lnz[:], dn[:])
                p0 = stp.tile([128, NU], F32, tag="p0")
                nc.vector.tensor_scalar(out=p0[:], in0=Hent[:], scalar1=POLY[0],
                                        scalar2=POLY[1], op0=ALU.mult, op1=ALU.add)
                p1 = stp.tile([128, NU], F32, tag="p1")
                for c in POLY[2:]:
                    nc.vector.tensor_mul(p1[:], p0[:], Hent[:])
                    nc.vector.tensor_scalar_add(p0[:], p1[:], c)
                nc.vector.tensor_scalar_max(p1[:], p0[:], 1.0)
                mk = stp.tile([128, NU], I32, tag="mk")
                nc.vector.tensor_scalar(out=mk[:], in0=Hent[:], scalar1=0.5,
                                        scalar2=None, op0=ALU.is_gt)
                nc.vector.tensor_copy(beta_all[:], ones64[:])
                nc.vector.copy_predicated(beta_all[:], mk[:], p1[:])

                # ---- B2: weighted softmax + attn @ v ----
                with tc.tile_pool(name="b2_ps", bufs=4, space="PSUM") as scps2, \
                     tc.tile_pool(name="tp_ps", bufs=3, space="PSUM") as tpps, \
                     tc.tile_pool(name="av_ps", bufs=1, space="PSUM") as avps_pool:
                    for rb in range(NRB):
                        W = 128 * (rb + 1)
                        njt = rb + 1
                        chunks = [(off, min(CHUNK, W - off)) for off in range(0, W, CHUNK)]
                        nck = len(chunks)
                        avp = None
                        for h in range(NH):
                            col = rb * NH + h
                            m, base = h // 2, 64 * (h % 2)
                            q_l = qT[m][base:base + 64, 128 * rb:128 * (rb + 1)]
                            z2c = stp.tile([128, 4], F32, tag="z2c")
                            z2s = stp.tile([128, 1], F32, tag="z2s")
                            t2 = t2p.tile([128, N], F32, tag="t2")
                            for ci, (off, cw) in enumerate(chunks):
                                ps2 = scps2.tile([128, CHUNK], F32, tag="ps_s")
                                _scores(nc, ps2, q_l, kT[m], base, off, cw)
                                if off + cw == W:
                                    nc.vector.tensor_tensor(
                                        out=ps2[:, cw - 128:cw], in0=ps2[:, cw - 128:cw],
                                        in1=mask[:], op=ALU.add)
                                nc.scalar.activation(
                                    t2[:, off:off + cw], ps2[:, :cw], AFT.Exp,
                                    bias=0.0, scale=beta_all[:, col:col + 1],
                                    accum_out=(z2c[:, ci:ci + 1] if nck > 1 else z2s[:]))
                            if nck > 1:
                                nc.vector.tensor_reduce(out=z2s[:], in_=z2c[:, :nck],
                                                        axis=mybir.AxisListType.X,
                                                        op=ALU.add)
                            rz2 = stp.tile([128, 1], F32, tag="rz2")
                            nc.vector.reciprocal(rz2[:], z2s[:])
                            t2bf = t2bfp.tile([128, N], BF16, tag="t2bf")
                            nc.vector.tensor_scalar_mul(t2bf[:, :W], t2[:, :W], rz2[:])

                            # transpose + av
                            if h % 2 == 0:
                                avp = avps_pool.tile([128, 128], F32, tag="avp")
                            for g in range(0, njt, 4):
                                gn = min(4, njt - g)
                                tp = tpps.tile([128, 512], BF16, tag="tp")
                                for kk in range(gn):
                                    jt = g + kk
                                    nc.tensor.transpose(
                                        tp[:, 128 * kk:128 * (kk + 1)],
                                        t2bf[:, 128 * jt:128 * (jt + 1)], ident[:])
                                tts = ttp.tile([128, 512], BF16, tag="tts")
                                nc.any.tensor_copy(tts[:, :128 * gn], tp[:, :128 * gn])
                                for kk in range(gn):
                                    jt = g + kk
                                    nc.tensor.matmul(
                                        avp[base:base + 64, :],
                                        v_bf[jt][:, 64 * h:64 * (h + 1)],
                                        tts[:, 128 * kk:128 * (kk + 1)],
                                        start=(jt == 0), stop=(jt == njt - 1),
                                        tile_position=(0, base))
                            if h % 2 == 1:
                                nc.any.tensor_copy(attT[m][:, 128 * rb:128 * (rb + 1)], avp[:])
                # ---- phase C: output projection ----
                with tc.tile_pool(name="pj_ps", bufs=2, space="PSUM") as pjps:
                    for rb in range(NRB):
                        for nn in range(2):
                            pp = pjps.tile([128, 512], F32, tag="pp")
                            for m in range(2):
                                nc.tensor.matmul(
                                    pp[:], attT[m][:, 128 * rb:128 * (rb + 1)],
                                    woS[m][:, 512 * nn:512 * (nn + 1)],
                                    start=(m == 0), stop=(m == 1))
                            ost = ostp.tile([128, 512], F32, tag="ost")
                            nc.any.tensor_copy(ost[:], pp[:])
                            nc.sync.dma_start(
                                partial[128 * rb:128 * (rb + 1), 512 * nn:512 * (nn + 1)],
                                ost[:])

    nc.compile()
    return nc


_NC_CACHE = None
_LAST_IN_MAPS = None


def kernel(x, Wq, Wk, Wv, Wo, bo):
    global _NC_CACHE, _LAST_IN_MAPS
    x = np.asarray(x, dtype=np.float32)
    Wq = np.asarray(Wq, dtype=np.float32)
    Wk = np.asarray(Wk, dtype=np.float32)
    Wv = np.asarray(Wv, dtype=np.float32)
    Wo = np.asarray(Wo, dtype=np.float32)
    bo = np.asarray(bo, dtype=np.float32)

    if _NC_CACHE is None:
        _NC_CACHE = build_kernel()
    nc = _NC_CACHE

    mask_h = np.where(np.arange(128)[None, :] > np.arange(128)[:, None],
                      np.float32(MASK_VAL), np.float32(0.0)).astype(np.float32)
    woT_full = np.ascontiguousarray(Wo.T)  # [c, o]

    in_maps = []
    for c in range(N_CORES):
        b = c // 4
        s0 = CD * (c % 4)
        sl = slice(s0, s0 + CD)
        in_maps.append({
            "xT": np.ascontiguousarray(x[b].T),
            "wqT": np.ascontiguousarray(Wq[sl, :].T),
            "wkT": np.ascontiguousarray(Wk[sl, :].T),
            "wvT": np.ascontiguousarray(Wv[sl, :].T),
            "woT": np.ascontiguousarray(woT_full[sl, :]),
            "maskin": mask_h,
        })

    _LAST_IN_MAPS = in_maps
    res = run_bass_kernel_spmd(nc, in_maps, core_ids=list(range(N_CORES)))

    out = np.zeros((B, N, DIM), dtype=np.float32)
    for c in range(N_CORES):
        out[c // 4] += res.results[c]["partial"]
    out += bo[None, None, :]
    return out



# revision 18
# speedup vs baseline: 1.0896x; 1.0896x over previous
"""Trainium2 Bass kernel for causal multi-head attention with adaptive
temperature (entropy-polynomial) softmax.

Problem shape: x [2, 2048, 1024], 16 heads x 64 dims, causal.
  q/k/v = x @ W{q,k,v}.T ; sim = q k^T / 8 (causal) ;
  attn = softmax(beta * sim), beta = f(entropy(softmax(sim))) ;
  out = (attn v) @ Wo.T + bo

Sharding (8 cores): core c owns batch b = c // 4 and heads
4*(c%4) .. 4*(c%4)+3.  Each core computes its heads' q/k/v projections,
full [n, n] score tiles for its heads, and a partial output projection
over its 256 channel dims.  Host sums the 4 partials per batch + bo.

Device-side structure (v2 - transposed second pass):
  phase A : qT/kT = (W slice) @ x^T in f32r, drained to bf16
            (qT pre-scaled 1/8); v -> v_aug [128, 4*65] bf16 with a
            ones column per head (folds Z2 into the AV matmul).
  phase B1: row-wise score sweep in 1024-wide chunks:
            scores (bf16 matmul) -> exp (ACT, no accum) ->
            Z1 = sum exp via Pool stt(t1*ones, accum) ;
            D = sum l*exp via DVE stt(ps*t1, accum).
  stats   : H = ln Z1 - D/Z1 ; beta = where(H>.5, max(poly(H),1), 1)
            computed batched [128, 64]; transposed via PE; folded into
            qbT = qT * beta (per-column broadcast via tiny PE matmul).
  phase B2: TRANSPOSED rescore: psT[j, r] = kT^T qb (512-wide over 4
            row blocks at once) -> exp -> t2T bf16 ->
            AV: avp[65, 512] += v_aug^T t2T  (row 64 = Z2).
            Normalize: attT = avp[0:64] * bcast(1/avp[64]).
  phase C : partial[r, o] = sum_m attT[m]^T woS[m]  (bf16)
"""

import numpy as np

import concourse.bass as bass
import concourse.tile as tile
from concourse import bacc, mybir
from concourse.bass_utils import run_bass_kernel_spmd
from concourse.masks import make_identity

F32 = mybir.dt.float32
F32R = mybir.dt.float32r
BF16 = mybir.dt.bfloat16
I32 = mybir.dt.int32
AFT = mybir.ActivationFunctionType
ALU = mybir.AluOpType

B, N, DIM = 2, 2048, 1024
H_TOT, HD = 16, 64
N_CORES = 8
NH = 4            # heads per core
CD = NH * HD      # 256 channel dims per core
NRB = N // 128    # 16 row blocks
NU = NRB * NH     # 64 (rb, head) units
NT = N // 512     # 4 row-groups of 512 rows
POLY = [-0.037, 0.481, -2.3, 4.917, -1.791]
MASK_VAL = -1e30
SCALE = 1.0 / 8.0  # 1/sqrt(64)

B1CHUNK = 1024    # B1 score-chunk width (2 PSUM banks)


def build_kernel():
    nc = bacc.Bacc("TRN2", target_bir_lowering=False, debug=False,
                   num_devices=N_CORES)

    xT = nc.dram_tensor("xT", [DIM, N], F32, kind="ExternalInput").ap()
    wqT = nc.dram_tensor("wqT", [DIM, CD], F32, kind="ExternalInput").ap()
    wkT = nc.dram_tensor("wkT", [DIM, CD], F32, kind="ExternalInput").ap()
    wvT = nc.dram_tensor("wvT", [DIM, CD], F32, kind="ExternalInput").ap()
    woT = nc.dram_tensor("woT", [CD, DIM], F32, kind="ExternalInput").ap()
    partial = nc.dram_tensor("partial", [N, DIM], F32, kind="ExternalOutput").ap()

    KC = DIM // 128  # 8 contraction chunks

    with tile.TileContext(nc) as tc:
        # ---- persistent pools ----
        with tc.tile_pool(name="const", bufs=1) as constp, \
             tc.tile_pool(name="qkv_sb", bufs=1) as qkvp, \
             tc.tile_pool(name="attn_out", bufs=1) as aop, \
             tc.tile_pool(name="wo_sb", bufs=1) as wop, \
             tc.tile_pool(name="statsall", bufs=1) as sap:

            identF = constp.tile([128, 128], F32)
            make_identity(nc, identF[:])
            # row mask: mask[r, j] = MASK_VAL if j > r else 0
            mask = constp.tile([128, 128], F32)
            nc.gpsimd.memset(mask[:], 0.0)
            nc.gpsimd.affine_select(
                out=mask[:], in_=mask[:], compare_op=ALU.is_ge,
                fill=MASK_VAL, base=0, pattern=[[-1, 128]],
                channel_multiplier=1)
            # transposed mask: maskT[j, r] = MASK_VAL if j > r else 0
            maskT = constp.tile([128, 128], F32)
            nc.gpsimd.memset(maskT[:], 0.0)
            nc.gpsimd.affine_select(
                out=maskT[:], in_=maskT[:], compare_op=ALU.is_ge,
                fill=MASK_VAL, base=0, pattern=[[1, 128]],
                channel_multiplier=-1)
            ones64 = constp.tile([128, NU], F32)
            nc.vector.memset(ones64[:], 1.0)
            identB = constp.tile([128, 128], BF16)
            make_identity(nc, identB[:])
            maskB = constp.tile([128, 128], BF16)
            nc.gpsimd.tensor_copy(maskB[:], mask[:])
            maskTB = constp.tile([128, 128], BF16)
            nc.gpsimd.tensor_copy(maskTB[:], maskT[:])

            # persistent activations (bf16)
            qT = [qkvp.tile([128, N], BF16, tag=f"qT{m}", name=f"qT{m}") for m in range(2)]
            kT = [qkvp.tile([128, N], BF16, tag=f"kT{m}", name=f"kT{m}") for m in range(2)]
            qbT = [qkvp.tile([128, N], BF16, tag=f"qbT{m}", name=f"qbT{m}") for m in range(2)]
            # v_aug: head h at cols 65h..65h+63, ones col at 65h+64
            v_aug = [qkvp.tile([128, NH * 65], BF16, tag=f"v{j}", name=f"v{j}")
                     for j in range(NRB)]
            attT = [aop.tile([128, N], BF16, tag=f"attT{m}", name=f"attT{m}") for m in range(2)]
            woS = [wop.tile([128, DIM], BF16, tag=f"wo{m}", name=f"wo{m}") for m in range(2)]

            Z1a = sap.tile([128, NU], F32)
            D1a = sap.tile([128, NU], F32)

            # ---- phase A: QKV projections (f32r in, bf16 out) ----
            with tc.tile_pool(name="xw_sb", bufs=1) as xwp, \
                 tc.tile_pool(name="qkv_ps", bufs=4, space="PSUM") as qkps:
                xTs = [xwp.tile([128, N], F32R, tag=f"xT{k}", name=f"xTs{k}") for k in range(KC)]
                wq_s = [xwp.tile([128, CD], F32R, tag=f"wq{k}", name=f"wq{k}") for k in range(KC)]
                wk_s = [xwp.tile([128, CD], F32R, tag=f"wk{k}", name=f"wk{k}") for k in range(KC)]
                wv_s = [xwp.tile([128, CD], F32R, tag=f"wv{k}", name=f"wv{k}") for k in range(KC)]
                woF = [xwp.tile([128, DIM], F32, tag=f"woF{m}", name=f"woF{m}") for m in range(2)]
                for k in range(KC):
                    sl = slice(128 * k, 128 * (k + 1))
                    nc.sync.dma_start(wq_s[k][:], wqT[sl, :].bitcast(F32R))
                    nc.sync.dma_start(wk_s[k][:], wkT[sl, :].bitcast(F32R))
                    nc.sync.dma_start(wv_s[k][:], wvT[sl, :].bitcast(F32R))
                    nc.sync.dma_start(xTs[k][:], xT[sl, :].bitcast(F32R))
                for m in range(2):
                    nc.sync.dma_start(woF[m][:], woT[128 * m:128 * (m + 1), :])
                    nc.gpsimd.tensor_copy(woS[m][:], woF[m][:])

                # qT / kT: [o, r] = sum_c W[o,c] x[r,c]; drain to bf16
                for m in range(2):
                    for which, wt, dest, scl in (("q", wq_s, qT, SCALE), ("k", wk_s, kT, 1.0)):
                        for nn in range(N // 512):
                            pq = qkps.tile([128, 512], F32, tag="pq")
                            for k in range(KC):
                                nc.tensor.matmul(
                                    pq[:], wt[k][:, 128 * m:128 * (m + 1)],
                                    xTs[k][:, 512 * nn:512 * (nn + 1)],
                                    start=(k == 0), stop=(k == KC - 1))
                            nc.scalar.activation(
                                dest[m][:, 512 * nn:512 * (nn + 1)], pq[:],
                                AFT.Copy, bias=0.0, scale=scl)

                # v: [j, d] = sum_c xT[c,j] wvT[c,d] -> strided into v_aug
                for jt in range(NRB):
                    pv = qkps.tile([128, CD], F32, tag="pv")
                    for k in range(KC):
                        nc.tensor.matmul(
                            pv[:], xTs[k][:, 128 * jt:128 * (jt + 1)], wv_s[k][:],
                            start=(k == 0), stop=(k == KC - 1))
                    nc.gpsimd.memset(v_aug[jt][:], 1.0)
                    nc.vector.tensor_copy(
                        v_aug[jt].rearrange("p (h d) -> p h d", d=65)[:, :, 0:64],
                        pv.rearrange("p (h d) -> p h d", d=64))

            # ---- phase B1: row-wise entropy-stat sweep ----
            # one [128, W] psum tile per (rb, h) unit; the causal mask is
            # folded into the score accumulation group as ident.T @ mask.
            with tc.tile_pool(name="b1_ps", bufs=2, space="PSUM") as scps, \
                 tc.tile_pool(name="t1p", bufs=3) as t1p, \
                 tc.tile_pool(name="scr2", bufs=2) as scr2:
                for rb in range(NRB):
                    W = 128 * (rb + 1)
                    for h in range(NH):
                        u = rb * NH + h
                        m, base = h // 2, 64 * (h % 2)
                        q_l = qT[m][base:base + 64, 128 * rb:128 * (rb + 1)]
                        ps = scps.tile([128, N], F32, tag="ps_s")
                        for o2 in range(0, W, 512):
                            sw = min(512, W - o2)
                            last = o2 + sw == W
                            nc.tensor.matmul(
                                ps[:, o2:o2 + sw], q_l,
                                kT[m][base:base + 64, o2:o2 + sw],
                                start=True, stop=not last,
                                skip_group_check=last)
                        nc.tensor.matmul(
                            ps[:, W - 128:W], identB[:], maskB[:],
                            start=False, stop=True, skip_group_check=True)
                        t1 = t1p.tile([128, N], BF16, tag="t1")
                        nc.scalar.activation(
                            t1[:, :W], ps[:, :W], AFT.Exp, bias=0.0, scale=1.0)
                        nc.vector.tensor_reduce(
                            out=Z1a[:, u:u + 1], in_=t1[:, :W],
                            axis=mybir.AxisListType.X, op=ALU.add)
                        s2 = scr2.tile([128, N], BF16, tag="s2")
                        nc.vector.scalar_tensor_tensor(
                            out=s2[:, :W], in0=ps[:, :W], scalar=1.0,
                            in1=t1[:, :W], op0=ALU.mult, op1=ALU.mult,
                            accum_out=D1a[:, u:u + 1])

            # ---- batched stats: H = ln Z1 - D/Z1 ; beta ; qbT ----
            with tc.tile_pool(name="stats", bufs=1) as stp, \
                 tc.tile_pool(name="bc_ps", bufs=4, space="PSUM") as bcps, \
                 tc.tile_pool(name="bcp", bufs=4) as bcp:
                rz = stp.tile([128, NU], F32, tag="rz")
                nc.vector.reciprocal(rz[:], Z1a[:])
                dn = stp.tile([128, NU], F32, tag="dn")
                nc.vector.tensor_mul(dn[:], D1a[:], rz[:])
                lnz = stp.tile([128, NU], F32, tag="lnz")
                nc.scalar.activation(lnz[:], Z1a[:], AFT.Ln, bias=0.0, scale=1.0)
                Hent = stp.tile([128, NU], F32, tag="Hent")
                nc.vector.tensor_sub(Hent[:], lnz[:], dn[:])
                p0 = stp.tile([128, NU], F32, tag="p0")
                nc.vector.tensor_scalar(out=p0[:], in0=Hent[:], scalar1=POLY[0],
                                        scalar2=POLY[1], op0=ALU.mult, op1=ALU.add)
                p1 = stp.tile([128, NU], F32, tag="p1")
                for c in POLY[2:]:
                    nc.vector.tensor_mul(p1[:], p0[:], Hent[:])
                    nc.vector.tensor_scalar_add(p0[:], p1[:], c)
                nc.vector.tensor_scalar_max(p1[:], p0[:], 1.0)
                mk = stp.tile([128, NU], I32, tag="mk")
                nc.vector.tensor_scalar(out=mk[:], in0=Hent[:], scalar1=0.5,
                                        scalar2=None, op0=ALU.is_gt)
                beta_all = stp.tile([128, NU], F32, tag="beta_all")
                nc.vector.tensor_copy(beta_all[:], ones64[:])
                nc.vector.copy_predicated(beta_all[:], mk[:], p1[:])

                # qbT = qT * bcast(beta): replicate the head pair's beta
                # columns 64x along free (stride-0 view), transpose on PE to
                # get [head-dim part, r], multiply into qbT.
                for m in range(2):
                    for rb in range(NRB):
                        u0 = 4 * rb + 2 * m
                        src = beta_all[:, u0:u0 + 2]
                        view = bass.AP(src.tensor, src.offset,
                                       [src.ap[0], src.ap[1], [0, 64]])
                        bcT = bcp.tile([128, 128], F32, tag="bcT")
                        nc.vector.tensor_copy(
                            bcT.rearrange("p (h r) -> p h r", r=64), view)
                        bc = bcps.tile([128, 128], F32, tag="bc")
                        nc.tensor.transpose(bc[:], bcT[:], identF[:])
                        cols = slice(128 * rb, 128 * (rb + 1))
                        nc.vector.tensor_tensor(
                            out=qbT[m][:, cols], in0=qT[m][:, cols],
                            in1=bc[:], op=ALU.mult)

            # ---- phase B2 (transposed rescore + AV) + phase C ----
            with tc.tile_pool(name="b2_ps", bufs=2, space="PSUM") as psTp, \
                 tc.tile_pool(name="t2p", bufs=3) as t2p, \
                 tc.tile_pool(name="av_ps", bufs=2, space="PSUM") as avpp, \
                 tc.tile_pool(name="rzp", bufs=2) as rzp, \
                 tc.tile_pool(name="pj_ps", bufs=2, space="PSUM") as pjps, \
                 tc.tile_pool(name="ost", bufs=2) as ostp:
                for t in range(NT):
                    njt = 4 * (t + 1)
                    rcols = slice(512 * t, 512 * (t + 1))
                    for h in range(NH):
                        m, base = h // 2, 64 * (h % 2)
                        qb_l = qbT[m][base:base + 64, rcols]
                        avp = avpp.tile([128, 512], F32, tag="avp")

                        def emit_av(j1, j2, t2, c1, c2):
                            for jj, cc, half in ((j1, c1, 0), (j2, c2, 1)):
                                lo = 128 * cc if cc > 0 else 0
                                nc.tensor.matmul(
                                    avp[0:65, lo:512],
                                    v_aug[jj][:, 65 * h:65 * h + 65],
                                    t2[:, 512 * half + lo:512 * (half + 1)],
                                    start=(jj == 0), stop=(jj == njt - 1),
                                    skip_group_check=True)

                        prev = None
                        for p in range(njt // 2):
                            j1, j2 = 2 * p, 2 * p + 1
                            c1, c2 = j1 - 4 * t, j2 - 4 * t
                            psT = psTp.tile([128, 1024], F32, tag="psT")
                            for jj, cc, half in ((j1, c1, 0), (j2, c2, 1)):
                                ing = cc >= 0
                                nc.tensor.matmul(
                                    psT[:, 512 * half:512 * (half + 1)],
                                    kT[m][base:base + 64, 128 * jj:128 * (jj + 1)],
                                    qb_l, start=True, stop=not ing,
                                    skip_group_check=ing)
                                if ing:
                                    dg = slice(512 * half + 128 * cc,
                                               512 * half + 128 * (cc + 1))
                                    nc.tensor.matmul(
                                        psT[:, dg], identB[:], maskTB[:],
                                        start=False, stop=True,
                                        skip_group_check=True)
                            lo = 128 * c1 if c1 > 0 else 0
                            t2 = t2p.tile([128, 1024], BF16, tag="t2")
                            nc.scalar.activation(
                                t2[:, lo:1024], psT[:, lo:1024], AFT.Exp,
                                bias=0.0, scale=1.0)
                            if prev is not None:
                                emit_av(*prev)
                            prev = (j1, j2, t2, c1, c2)
                        emit_av(*prev)

                        # normalize by Z2 (row 64 of avp)
                        rz2 = rzp.tile([128, 512], F32, tag="rz2")
                        nc.vector.reciprocal(rz2[0:1, :], avp[64:65, :])
                        rbc = rzp.tile([128, 512], F32, tag="rbc")
                        nc.gpsimd.partition_broadcast(rbc[0:64, :], rz2[0:1, :])
                        nc.vector.tensor_tensor(
                            out=attT[m][base:base + 64, rcols],
                            in0=avp[0:64, :], in1=rbc[0:64, :], op=ALU.mult)

                    # phase C for this row-group
                    for rb in range(4 * t, 4 * t + 4):
                        for nn in range(2):
                            pp = pjps.tile([128, 512], F32, tag="pp")
                            for m in range(2):
                                nc.tensor.matmul(
                                    pp[:], attT[m][:, 128 * rb:128 * (rb + 1)],
                                    woS[m][:, 512 * nn:512 * (nn + 1)],
                                    start=(m == 0), stop=(m == 1))
                            ost = ostp.tile([128, 512], F32, tag="ost")
                            nc.scalar.copy(ost[:], pp[:])
                            nc.sync.dma_start(
                                partial[128 * rb:128 * (rb + 1),
                                        512 * nn:512 * (nn + 1)],
                                ost[:])

    nc.compile()
    return nc


_NC_CACHE = None
_LAST_IN_MAPS = None


def kernel(x, Wq, Wk, Wv, Wo, bo):
    global _NC_CACHE, _LAST_IN_MAPS
    x = np.asarray(x, dtype=np.float32)
    Wq = np.asarray(Wq, dtype=np.float32)
    Wk = np.asarray(Wk, dtype=np.float32)
    Wv = np.asarray(Wv, dtype=np.float32)
    Wo = np.asarray(Wo, dtype=np.float32)
    bo = np.asarray(bo, dtype=np.float32)

    if _NC_CACHE is None:
        _NC_CACHE = build_kernel()
    nc = _NC_CACHE

    woT_full = np.ascontiguousarray(Wo.T)  # [c, o]

    in_maps = []
    for c in range(N_CORES):
        b = c // 4
        s0 = CD * (c % 4)
        sl = slice(s0, s0 + CD)
        in_maps.append({
            "xT": np.ascontiguousarray(x[b].T),
            "wqT": np.ascontiguousarray(Wq[sl, :].T),
            "wkT": np.ascontiguousarray(Wk[sl, :].T),
            "wvT": np.ascontiguousarray(Wv[sl, :].T),
            "woT": np.ascontiguousarray(woT_full[sl, :]),
        })

    _LAST_IN_MAPS = in_maps
    res = run_bass_kernel_spmd(nc, in_maps, core_ids=list(range(N_CORES)))

    out = np.zeros((B, N, DIM), dtype=np.float32)
    for c in range(N_CORES):
        out[c // 4] += res.results[c]["partial"]
    out += bo[None, None, :]
    return out


# revision 21
# speedup vs baseline: 1.1340x; 1.0407x over previous
"""Trainium2 Bass kernel for causal multi-head attention with adaptive
temperature (entropy-polynomial) softmax.

Problem shape: x [2, 2048, 1024], 16 heads x 64 dims, causal.
  q/k/v = x @ W{q,k,v}.T ; sim = q k^T / 8 (causal) ;
  attn = softmax(beta * sim), beta = f(entropy(softmax(sim))) ;
  out = (attn v) @ Wo.T + bo

Sharding (8 cores): core c owns batch b = c // 4 and heads
4*(c%4) .. 4*(c%4)+3.  Host sums the 4 partials per batch + bo.

v4 structure - software-pipelined over 512-row groups t so the PE stays
continuously busy (TRN2 HAM clock gate halves the PE clock when idle):

  phase A : qT/kT = (W slice) @ x^T in f32r, drained to bf16
            (qT pre-scaled 1/8); v -> v_aug [128, 4*65] bf16 with a
            ones column per head (folds Z2 into the AV matmul).
  B1 unit (rb, h): row-wise scores in <=1024-wide chunks -> exp (ACT)
            -> causal diag zeroed on the exp VALUES via gpsimd
            affine_select -> Z1 (ACT accum for non-diag chunks, DVE
            reduce for diag chunks) ; D via DVE stt accum.
  stats(t): H = ln Z1 - D/Z1 ; beta = where(H>.5, max(poly(H),1), 1)
            on [128, 16] unit slices; beta folded into qbT = qT * beta
            (stride-0 replicate + PE transpose + DVE mult).
  B2(t, h): TRANSPOSED rescore psT[j, r] = kT^T qb in jt-pairs ->
            exp -> t2 bf16 -> diag zeroed (gpsimd) ->
            AV: avp[65, 512] += v_aug^T t2  (row 64 = Z2) ->
            attT = avp[0:64] * bcast(1/avp[64]).
  C(t)    : partial[r, o] = sum_m attT[m]^T woS[m]  (bf16)

  Main loop: for t: { B2(t, h) + B1 units of group t+1 interleaved;
  stats(t+1); C(t) } - PE alternates B2/B1 matmuls without gaps while
  ACT/DVE/Pool drain the elementwise chains behind it.
"""

import numpy as np

import concourse.bass as bass
import concourse.tile as tile
from concourse import bacc, mybir
from concourse.bass_utils import run_bass_kernel_spmd
from concourse.masks import make_identity

F32 = mybir.dt.float32
F32R = mybir.dt.float32r
BF16 = mybir.dt.bfloat16
I32 = mybir.dt.int32
AFT = mybir.ActivationFunctionType
ALU = mybir.AluOpType

B, N, DIM = 2, 2048, 1024
H_TOT, HD = 16, 64
N_CORES = 8
NH = 4            # heads per core
CD = NH * HD      # 256 channel dims per core
NRB = N // 128    # 16 row blocks
NU = NRB * NH     # 64 (rb, head) units
NT = N // 512     # 4 row-groups of 512 rows
POLY = [-0.037, 0.481, -2.3, 4.917, -1.791]
SCALE = 1.0 / 8.0  # 1/sqrt(64)


def build_kernel():
    nc = bacc.Bacc("TRN2", target_bir_lowering=False, debug=False,
                   num_devices=N_CORES)

    xT = nc.dram_tensor("xT", [DIM, N], F32, kind="ExternalInput").ap()
    wqT = nc.dram_tensor("wqT", [DIM, CD], F32, kind="ExternalInput").ap()
    wkT = nc.dram_tensor("wkT", [DIM, CD], F32, kind="ExternalInput").ap()
    wvT = nc.dram_tensor("wvT", [DIM, CD], F32, kind="ExternalInput").ap()
    woT = nc.dram_tensor("woT", [CD, DIM], F32, kind="ExternalInput").ap()
    partial = nc.dram_tensor("partial", [N, DIM], F32, kind="ExternalOutput").ap()

    KC = DIM // 128  # 8 contraction chunks

    with tile.TileContext(nc) as tc:
        # ---- persistent pools ----
        with tc.tile_pool(name="const", bufs=1) as constp, \
             tc.tile_pool(name="qkv_sb", bufs=1) as qkvp, \
             tc.tile_pool(name="attn_out", bufs=1) as aop, \
             tc.tile_pool(name="wo_sb", bufs=1) as wop, \
             tc.tile_pool(name="statsall", bufs=1) as sap:

            identF = constp.tile([128, 128], F32)
            make_identity(nc, identF[:])
            ones64 = constp.tile([128, NU], F32)
            nc.vector.memset(ones64[:], 1.0)

            # persistent activations (bf16)
            qT = [qkvp.tile([128, N], BF16, tag=f"qT{m}", name=f"qT{m}") for m in range(2)]
            kT = [qkvp.tile([128, N], BF16, tag=f"kT{m}", name=f"kT{m}") for m in range(2)]
            qbT = [qkvp.tile([128, N], BF16, tag=f"qbT{m}", name=f"qbT{m}") for m in range(2)]
            v_aug = [qkvp.tile([128, NH * 65], BF16, tag=f"v{j}", name=f"v{j}")
                     for j in range(NRB)]
            attT = [aop.tile([128, N], BF16, tag=f"attT{m}", name=f"attT{m}") for m in range(2)]
            woS = [wop.tile([128, DIM], BF16, tag=f"wo{m}", name=f"wo{m}") for m in range(2)]

            Z1p = sap.tile([128, 2 * NU], F32)
            D1p = sap.tile([128, 2 * NU], F32)
            Z1a = sap.tile([128, NU], F32)
            D1a = sap.tile([128, NU], F32)
            beta_all = sap.tile([128, NU], F32)
            st_rz = sap.tile([128, NU], F32)
            st_dn = sap.tile([128, NU], F32)
            st_ln = sap.tile([128, NU], F32)
            st_H = sap.tile([128, NU], F32)
            st_p0 = sap.tile([128, NU], F32)
            st_p1 = sap.tile([128, NU], F32)
            st_mk = sap.tile([128, NU], I32)

            # ---- phase A: QKV projections (f32r in, bf16 out) ----
            with tc.tile_pool(name="xw_sb", bufs=1) as xwp, \
                 tc.tile_pool(name="qkv_ps", bufs=4, space="PSUM") as qkps:
                xTs = [xwp.tile([128, N], F32R, tag=f"xT{k}", name=f"xTs{k}") for k in range(KC)]
                wq_s = [xwp.tile([128, CD], F32R, tag=f"wq{k}", name=f"wq{k}") for k in range(KC)]
                wk_s = [xwp.tile([128, CD], F32R, tag=f"wk{k}", name=f"wk{k}") for k in range(KC)]
                wv_s = [xwp.tile([128, CD], F32R, tag=f"wv{k}", name=f"wv{k}") for k in range(KC)]
                woF = [xwp.tile([128, DIM], F32, tag=f"woF{m}", name=f"woF{m}") for m in range(2)]
                # q/k inputs first so the first projections start early
                for k in range(KC):
                    sl = slice(128 * k, 128 * (k + 1))
                    nc.sync.dma_start(wq_s[k][:], wqT[sl, :].bitcast(F32R))
                    nc.sync.dma_start(wk_s[k][:], wkT[sl, :].bitcast(F32R))
                    nc.sync.dma_start(xTs[k][:], xT[sl, :].bitcast(F32R))
                for k in range(KC):
                    sl = slice(128 * k, 128 * (k + 1))
                    nc.sync.dma_start(wv_s[k][:], wvT[sl, :].bitcast(F32R))
                for m in range(2):
                    nc.sync.dma_start(woF[m][:], woT[128 * m:128 * (m + 1), :])
                    nc.gpsimd.tensor_copy(woS[m][:], woF[m][:])

                for m in range(2):
                    for which, wt, dest, scl in (("q", wq_s, qT, SCALE), ("k", wk_s, kT, 1.0)):
                        for nn in range(N // 512):
                            pq = qkps.tile([128, 512], F32, tag="pq")
                            for k in range(KC):
                                nc.tensor.matmul(
                                    pq[:], wt[k][:, 128 * m:128 * (m + 1)],
                                    xTs[k][:, 512 * nn:512 * (nn + 1)],
                                    start=(k == 0), stop=(k == KC - 1))
                            nc.scalar.activation(
                                dest[m][:, 512 * nn:512 * (nn + 1)], pq[:],
                                AFT.Copy, bias=0.0, scale=scl)

                for jt in range(NRB):
                    pv = qkps.tile([128, CD], F32, tag="pv")
                    for k in range(KC):
                        nc.tensor.matmul(
                            pv[:], xTs[k][:, 128 * jt:128 * (jt + 1)], wv_s[k][:],
                            start=(k == 0), stop=(k == KC - 1))
                    nc.gpsimd.memset(v_aug[jt][:], 1.0)
                    nc.vector.tensor_copy(
                        v_aug[jt].rearrange("p (h d) -> p h d", d=65)[:, :, 0:64],
                        pv.rearrange("p (h d) -> p h d", d=64))

            nc.vector.memset(Z1p[:], 0.0)
            nc.vector.memset(D1p[:], 0.0)

            # ---- pipelined B1 / stats / B2 / C ----
            with tc.tile_pool(name="sc_ps", bufs=3, space="PSUM") as scp, \
                 tc.tile_pool(name="av_ps", bufs=2, space="PSUM") as avpp, \
                 tc.tile_pool(name="t1p", bufs=3) as t1p, \
                 tc.tile_pool(name="scr2", bufs=2) as scr2, \
                 tc.tile_pool(name="t2p", bufs=3) as t2p, \
                 tc.tile_pool(name="rzp", bufs=2) as rzp, \
                 tc.tile_pool(name="bcp", bufs=4) as bcp, \
                 tc.tile_pool(name="ostp", bufs=3) as ostp:

                def emit_b1_unit(rb, h):
                    """Row-wise stat sweep for unit (rb, h): Z1, D."""
                    u = rb * NH + h
                    m, base = h // 2, 64 * (h % 2)
                    q_l = qT[m][base:base + 64, 128 * rb:128 * (rb + 1)]
                    W = 128 * (rb + 1)
                    for ci, off in enumerate(range(0, W, 1024)):
                        cw = min(1024, W - off)
                        has_diag = off + cw == W
                        ps = scp.tile([128, 1024], F32, tag="sc")
                        for o2 in range(0, cw, 512):
                            sw = min(512, cw - o2)
                            nc.tensor.matmul(
                                ps[:, o2:o2 + sw], q_l,
                                kT[m][base:base + 64, off + o2:off + o2 + sw],
                                start=True, stop=True)
                        t1 = t1p.tile([128, 1024], BF16, tag="t1")
                        if has_diag:
                            nc.scalar.activation(
                                t1[:, :cw], ps[:, :cw], AFT.Exp,
                                bias=0.0, scale=1.0)
                            # zero exp values at masked (j > r) diag entries
                            nc.gpsimd.affine_select(
                                out=t1[:, cw - 128:cw], in_=t1[:, cw - 128:cw],
                                compare_op=ALU.is_ge, fill=0.0, base=0,
                                pattern=[[-1, 128]], channel_multiplier=1)
                            nc.vector.tensor_reduce(
                                out=Z1p[:, 2 * u + ci:2 * u + ci + 1],
                                in_=t1[:, :cw],
                                axis=mybir.AxisListType.X, op=ALU.add)
                        else:
                            nc.scalar.activation(
                                t1[:, :cw], ps[:, :cw], AFT.Exp,
                                bias=0.0, scale=1.0,
                                accum_out=Z1p[:, 2 * u + ci:2 * u + ci + 1])
                        s2 = scr2.tile([128, 1024], BF16, tag="s2")
                        nc.vector.scalar_tensor_tensor(
                            out=s2[:, :cw], in0=ps[:, :cw], scalar=1.0,
                            in1=t1[:, :cw], op0=ALU.mult, op1=ALU.mult,
                            accum_out=D1p[:, 2 * u + ci:2 * u + ci + 1])

                def emit_stats_qb(t):
                    """beta for units of row-group t, fold into qbT."""
                    us = slice(16 * t, 16 * (t + 1))
                    nc.vector.tensor_reduce(
                        out=Z1a[:, us],
                        in_=Z1p[:, 32 * t:32 * (t + 1)].rearrange(
                            "p (u c) -> p u c", c=2),
                        axis=mybir.AxisListType.X, op=ALU.add)
                    nc.vector.tensor_reduce(
                        out=D1a[:, us],
                        in_=D1p[:, 32 * t:32 * (t + 1)].rearrange(
                            "p (u c) -> p u c", c=2),
                        axis=mybir.AxisListType.X, op=ALU.add)
                    nc.vector.reciprocal(st_rz[:, us], Z1a[:, us])
                    nc.vector.tensor_mul(st_dn[:, us], D1a[:, us], st_rz[:, us])
                    nc.scalar.activation(st_ln[:, us], Z1a[:, us], AFT.Ln,
                                         bias=0.0, scale=1.0)
                    nc.vector.tensor_sub(st_H[:, us], st_ln[:, us], st_dn[:, us])
                    nc.vector.tensor_scalar(
                        out=st_p0[:, us], in0=st_H[:, us], scalar1=POLY[0],
                        scalar2=POLY[1], op0=ALU.mult, op1=ALU.add)
                    for c in POLY[2:]:
                        nc.vector.tensor_mul(st_p1[:, us], st_p0[:, us], st_H[:, us])
                        nc.vector.tensor_scalar_add(st_p0[:, us], st_p1[:, us], c)
                    nc.vector.tensor_scalar_max(st_p1[:, us], st_p0[:, us], 1.0)
                    nc.vector.tensor_scalar(out=st_mk[:, us], in0=st_H[:, us],
                                            scalar1=0.5, scalar2=None,
                                            op0=ALU.is_gt)
                    nc.vector.tensor_copy(beta_all[:, us], ones64[:, us])
                    nc.vector.copy_predicated(beta_all[:, us], st_mk[:, us],
                                              st_p1[:, us])
                    # qbT = qT * bcast(beta)
                    for g in range(2):  # two quads of (m, rb) pairs
                        bc4 = avpp.tile([128, 512], F32, tag="avp")
                        pairs = []
                        for i in range(4):
                            idx = 4 * g + i
                            m, rb = idx % 2, 4 * t + idx // 2
                            u0 = 4 * rb + 2 * m
                            src = beta_all[:, u0:u0 + 2]
                            view = bass.AP(src.tensor, src.offset,
                                           [src.ap[0], src.ap[1], [0, 64]])
                            bcT = bcp.tile([128, 128], F32, tag="bcT")
                            nc.vector.tensor_copy(
                                bcT.rearrange("p (h r) -> p h r", r=64), view)
                            nc.tensor.transpose(
                                bc4[:, 128 * i:128 * (i + 1)], bcT[:], identF[:])
                            pairs.append((m, rb, i))
                        for m, rb, i in pairs:
                            cols = slice(128 * rb, 128 * (rb + 1))
                            nc.vector.tensor_tensor(
                                out=qbT[m][:, cols], in0=qT[m][:, cols],
                                in1=bc4[:, 128 * i:128 * (i + 1)], op=ALU.mult)

                def emit_b2_head(t, h):
                    """Transposed rescore + AV + normalize for (t, h)."""
                    njt = 4 * (t + 1)
                    rcols = slice(512 * t, 512 * (t + 1))
                    m, base = h // 2, 64 * (h % 2)
                    qb_l = qbT[m][base:base + 64, rcols]
                    avp = avpp.tile([128, 512], F32, tag="avp")

                    def emit_av(j1, j2, t2, c1, c2):
                        for jj, cc, half in ((j1, c1, 0), (j2, c2, 1)):
                            lo = 128 * cc if cc > 0 else 0
                            nc.tensor.matmul(
                                avp[0:65, lo:512],
                                v_aug[jj][:, 65 * h:65 * h + 65],
                                t2[:, 512 * half + lo:512 * (half + 1)],
                                start=(jj == 0), stop=(jj == njt - 1),
                                skip_group_check=True)

                    prev = None
                    for p in range(njt // 2):
                        j1, j2 = 2 * p, 2 * p + 1
                        c1, c2 = j1 - 4 * t, j2 - 4 * t
                        psT = scp.tile([128, 1024], F32, tag="sc")
                        for jj, half in ((j1, 0), (j2, 1)):
                            nc.tensor.matmul(
                                psT[:, 512 * half:512 * (half + 1)],
                                kT[m][base:base + 64, 128 * jj:128 * (jj + 1)],
                                qb_l, start=True, stop=True)
                        lo = 128 * c1 if c1 > 0 else 0
                        t2 = t2p.tile([128, 1024], BF16, tag="t2")
                        nc.scalar.activation(
                            t2[:, lo:1024], psT[:, lo:1024], AFT.Exp,
                            bias=0.0, scale=1.0)
                        for cc, half in ((c1, 0), (c2, 1)):
                            if cc >= 0:
                                dg = slice(512 * half + 128 * cc,
                                           512 * half + 128 * (cc + 1))
                                nc.gpsimd.affine_select(
                                    out=t2[:, dg], in_=t2[:, dg],
                                    compare_op=ALU.is_ge, fill=0.0, base=0,
                                    pattern=[[1, 128]], channel_multiplier=-1)
                        if prev is not None:
                            emit_av(*prev)
                        prev = (j1, j2, t2, c1, c2)
                    emit_av(*prev)

                    # normalize by Z2 (row 64 of avp)
                    rz2 = rzp.tile([128, 512], F32, tag="rz2")
                    nc.vector.reciprocal(rz2[0:1, :], avp[64:65, :])
                    rbc = rzp.tile([128, 512], F32, tag="rbc")
                    nc.gpsimd.partition_broadcast(rbc[0:64, :], rz2[0:1, :])
                    nc.vector.tensor_tensor(
                        out=attT[m][base:base + 64, rcols],
                        in0=avp[0:64, :], in1=rbc[0:64, :], op=ALU.mult)

                def emit_c(t):
                    for rb in range(4 * t, 4 * t + 4):
                        for nn in range(2):
                            pp = avpp.tile([128, 512], F32, tag="avp")
                            for m in range(2):
                                nc.tensor.matmul(
                                    pp[:], attT[m][:, 128 * rb:128 * (rb + 1)],
                                    woS[m][:, 512 * nn:512 * (nn + 1)],
                                    start=(m == 0), stop=(m == 1))
                            ost = ostp.tile([128, 512], F32, tag="ost")
                            nc.vector.tensor_copy(ost[:], pp[:])
                            nc.sync.dma_start(
                                partial[128 * rb:128 * (rb + 1),
                                        512 * nn:512 * (nn + 1)],
                                ost[:])

                # prologue: B1 for row-group 0
                for rb in range(4):
                    for h in range(NH):
                        emit_b1_unit(rb, h)
                emit_stats_qb(0)

                for t in range(NT):
                    for h in range(NH):
                        emit_b2_head(t, h)
                        if t + 1 < NT:
                            for j in range(4):
                                emit_b1_unit(4 * (t + 1) + j, h)
                    if t + 1 < NT:
                        emit_stats_qb(t + 1)
                    emit_c(t)

    nc.compile()
    return nc


_NC_CACHE = None
_LAST_IN_MAPS = None


def kernel(x, Wq, Wk, Wv, Wo, bo):
    global _NC_CACHE, _LAST_IN_MAPS
    x = np.asarray(x, dtype=np.float32)
    Wq = np.asarray(Wq, dtype=np.float32)
    Wk = np.asarray(Wk, dtype=np.float32)
    Wv = np.asarray(Wv, dtype=np.float32)
    Wo = np.asarray(Wo, dtype=np.float32)
    bo = np.asarray(bo, dtype=np.float32)

    if _NC_CACHE is None:
        _NC_CACHE = build_kernel()
    nc = _NC_CACHE

    woT_full = np.ascontiguousarray(Wo.T)  # [c, o]

    in_maps = []
    for c in range(N_CORES):
        b = c // 4
        s0 = CD * (c % 4)
        sl = slice(s0, s0 + CD)
        in_maps.append({
            "xT": np.ascontiguousarray(x[b].T),
            "wqT": np.ascontiguousarray(Wq[sl, :].T),
            "wkT": np.ascontiguousarray(Wk[sl, :].T),
            "wvT": np.ascontiguousarray(Wv[sl, :].T),
            "woT": np.ascontiguousarray(woT_full[sl, :]),
        })

    _LAST_IN_MAPS = in_maps
    res = run_bass_kernel_spmd(nc, in_maps, core_ids=list(range(N_CORES)))

    out = np.zeros((B, N, DIM), dtype=np.float32)
    for c in range(N_CORES):
        out[c // 4] += res.results[c]["partial"]
    out += bo[None, None, :]
    return out
